# revision 43
# baseline (speedup 1.0000x reference)
"""Trainium2 Bass kernel for nn_AttentionPool (gnn_message_passing).

Strategy (v5: single-copy streaming, on-chip transpose, flipped pooling)
-----------------------------------------------------------------------
Math restructure (exactly equivalent to the reference up to fp rounding):
  score[n,h] = context_h[n,:] @ V[:,h] + c[h]        (V, c host-folded from
               W_lin/W_att/b_lin/b_att: V[k,h] = sum_o W_lin[h*128+o,k]W_att[o],
               c[h] = b_lin[h*128:(h+1)*128].W_att + b_att)
  p = exp(leaky_relu(score, 0.2))                     (skip segment-max: scores
               are O(1) so exp cannot overflow; softmax is shift-invariant)
  denom[cls,h]  = sum_{n: y=cls} p[n,h]
  pooled[cls,h,:] = sum_{n: y=cls} p[n,h]*context_h[n,:] / denom[cls,h]

Sharding: BY CLASS. Host argsorts context_y; core k owns classes
[125k, 125k+125) -> no cross-core reduction.

v4 sent context_h twice (node-major hn for pooling + feature-major htr for
the score matmul): 13.2MB/core of HBM traffic -> DMA-starved, PE idle 20us,
HAM clock oscillation. v5 sends hn ONCE (6.8MB/core) and derives the
feature-major copy on-chip:

  per tile j (128 nodes):
    PE transpose-mode:  htr_psum[c] = hn[:, j, 128c:128c+128].T   (bf16 PSUM)
    ACT/DVE copy:       htr_sb <- htr_psum                        (batched x4)
    score MMs:          s[n,h] += htr_sb[c].T @ V[c]  (+ c bias via ones-row
                        outer-product MM); LDWEIGHTS is FWL (4x) so the
                        8-col-moving MMs are cheap
    ACT:                slr = Lrelu(s, alpha=0.2); prep = Exp(slr)
    GPSIMD:             mask[n,w] = (yg[n]==w);  ow[n,h,w] = mask*prep
    PE pooling (FLIPPED, ow stationary):
        acc[hw, 0:258] += ow[:,j].T @ hn[:, j, 0:258]
      -> one matmul per tile; col 256 is yg (garbage, ignored), col 257 is
         1.0 so the denom row rides along for free (pad rows have ow=0).

Classes split into 8 groups of <=16 consecutive classes per core (GCLS=16);
each group's sorted nodes padded to whole 128-node tiles. acc is one PSUM
region [64, 258] per group, flushed by ACT and DMA'd out at group end.

PE program order per supertile i:  score(i) | transp(i+1) interleaved with
pool(i) | ... so ACT/DVE copies and GPSIMD ow-building overlap PE work.
"""

import sys

sys.path.insert(0, "/opt/trn_rl_repo")

import numpy as np
import ml_dtypes

BF = ml_dtypes.bfloat16

N = 100000
INC = 256
NHEAD = 4
OUTC = 128
NCLS = 1000
NCORES = 8
GCLS = 12  # class slots per group window
NGRP = 11  # groups (bins) per core
GCAP = 9 * 128  # target bin size: 9 tiles of nodes
HWW = NHEAD * GCLS  # 48: pooling moving width
NCOL = INC + 1  # hn row: 256 features, yg
OUTW = 3 * HWW  # per-group output block [128, 144]: c0, c1, denom row
OBATCH = 2  # groups per output DMA

_PROG_CACHE = {}
LAST_RESULT = None
LAST_PROFILE = None


def _schedule(t_tiles, s_tiles=16):
    """Supertile sizes (small first to prime the pipeline) and the per-
    supertile count of htr tiles sourced via DMA instead of PE transpose."""
    sizes = []
    rem = t_tiles
    for sz in (8,):
        if rem > sz + s_tiles:
            sizes.append(sz)
            rem -= sz
    while rem > 0:
        sizes.append(min(s_tiles, rem))
        rem -= sizes[-1]
    # NOTE: sourcing some htr tiles via DMA was tried (v7/v7b) and REGRESSED:
    # the steady state is DMA-paced at ~340GB/s effective, so any extra HBM
    # bytes stall the pipeline and HAM cold-clock amplifies the stalls.
    # All htr tiles are built on-chip by PE transposes.
    htr_c = [0] * len(sizes)
    return sizes, htr_c


def build_program(tg, s_tiles=16):
    """Build + compile the SPMD Bass program. tg = tiles per group (len 8)."""
    from concourse import bacc, mybir, tile

    f32 = mybir.dt.float32
    bf16 = mybir.dt.bfloat16
    AF = mybir.ActivationFunctionType
    OP = mybir.AluOpType

    t_tiles = int(sum(tg))
    grp_of = []
    for g, tcnt in enumerate(tg):
        for i in range(tcnt):
            grp_of.append((g, i == 0, i == tcnt - 1))

    nc = bacc.Bacc(
        "TRN2", target_bir_lowering=False, debug=False, num_devices=NCORES
    )

    hn = nc.dram_tensor("hn", [128, t_tiles, NCOL], bf16, kind="ExternalInput").ap()
    # consts: cols 0:16 iota, 32:40 [V|0.2V] chunk0, 40:48 chunk1,
    # row0 48:56 [c|0.2c], cols 64:192 identity
    cst = nc.dram_tensor("cst", [128, 192], bf16, kind="ExternalInput").ap()
    out = nc.dram_tensor(
        "outp", [128, NGRP * OUTW], f32, kind="ExternalOutput"
    ).ap()

    # supertile schedule + mixed-source htr: the first htr_c[s] tiles of each
    # supertile get their feature-major copy via DMA (spare HBM bandwidth,
    # shorter dep chain — full coverage early to prime the pipeline); the
    # rest are built on-chip by PE transposes (spare PE cycles)
    sizes, htr_c = _schedule(t_tiles, s_tiles)
    starts = []
    acc0 = 0
    for sz in sizes:
        starts.append(acc0)
        acc0 += sz
    nsup = len(sizes)

    def sup_range(s_):
        return starts[s_], sizes[s_]

    htr_off = []
    o = 0
    for s in range(nsup):
        htr_off.append(o)
        o += htr_c[s]
    ct_tiles = o
    htrd = None
    if ct_tiles > 0:
        htrd = nc.dram_tensor(
            "htrd", [128, ct_tiles, 2, 128], bf16, kind="ExternalInput"
        ).ap()

    with tile.TileContext(nc) as tc:
        with (
            tc.tile_pool(name="const", bufs=1) as cpool,
            tc.tile_pool(name="stream", bufs=6) as sb,
            tc.tile_pool(name="htrp", bufs=2) as htp,
            tc.tile_pool(name="owp", bufs=3) as owp,
            tc.tile_pool(name="work", bufs=3) as sg,
            tc.tile_pool(name="trp", bufs=4, space="PSUM") as trp,
            tc.tile_pool(name="sps", bufs=2, space="PSUM") as ps,
            tc.tile_pool(name="acc", bufs=2, space="PSUM") as accp,
        ):
            # ---- constants --------------------------------------------------
            cst_sb = cpool.tile([128, 192], bf16)
            nc.scalar.dma_start(out=cst_sb[:], in_=cst)
            ciota_sb = cst_sb[:, 0:GCLS]
            v_bf = [cst_sb[:, 32:40], cst_sb[:, 40:48]]
            ident = cst_sb[:, 64:192]
            ones_row = cpool.tile([1, 128], bf16)
            nc.vector.memset(ones_row[:1], 1.0)
            ones_col = cpool.tile([128, 1], bf16)
            nc.vector.memset(ones_col[:], 1.0)
            c_rep = cpool.tile([1, s_tiles * 2 * NHEAD], bf16)
            nc.vector.tensor_copy(
                out=c_rep[:1].rearrange("p (j h) -> p j h", h=2 * NHEAD),
                in_=cst_sb[0:1, 48:56]
                .to_broadcast([1, 2 * NHEAD, s_tiles])
                .rearrange("p h j -> p j h"),
            )

            out_sb = cpool.tile([128, NGRP * OUTW], f32)

            # HAM warm-up: ~15 dummy outer-product MMs depending only on the
            # ones_row memset, issued while the first DMAs stream, so the PE
            # clock (4/8 throttled when idle) is at full rate for real work
            wps = accp.tile(
                [128, OUTW], f32, tag="gacc", name="wps",
                padded_shape=[128, 512],
            )
            for _ in range(34):
                nc.tensor.matmul(
                    wps[:, 0:128], lhsT=ones_row[:1], rhs=ones_row[:1],
                    start=True, stop=True, skip_group_check=True,
                )

            hns_tiles = {}
            htr_tiles = {}
            ow_tiles = {}
            acc_state = {"t": None}
            copy_rr = {"i": 0}

            def load(s_):
                # two half-loads: range-aware deps let the first transp batch
                # start as soon as the first half lands
                ts0, nst = sup_range(s_)
                hns = sb.tile([128, s_tiles, NCOL], bf16, tag="hn", name="hns")
                h = min(8, nst)
                nc.sync.dma_start(
                    out=hns[:, :h, :], in_=hn[:, ts0 : ts0 + h, :]
                )
                if nst > h:
                    nc.sync.dma_start(
                        out=hns[:, h:nst, :], in_=hn[:, ts0 + h : ts0 + nst, :]
                    )
                hns_tiles[s_] = hns

            def mask_ow(s_, prep):
                # DVE: mask then ow (TensorTensor is illegal on Pool/GPSIMD)
                ts0, nst = sup_range(s_)
                hns = hns_tiles[s_]
                mask = sg.tile([128, s_tiles, GCLS], bf16, tag="mask", name="mask")
                nc.vector.tensor_tensor(
                    out=mask[:, :nst, :],
                    in0=hns[:, :nst, INC].to_broadcast([128, nst, GCLS]),
                    in1=ciota_sb.to_broadcast([128, GCLS, nst]).rearrange(
                        "p w j -> p j w"
                    ),
                    op=OP.is_equal,
                )
                ow = owp.tile([128, s_tiles, NHEAD, GCLS], bf16, tag="ow", name="ow")
                nc.vector.tensor_tensor(
                    out=ow[:, :nst],
                    in0=mask[:, :nst, :]
                    .to_broadcast([128, nst, GCLS, NHEAD])
                    .rearrange("p j w h -> p j h w"),
                    in1=prep[:, : nst * NHEAD]
                    .rearrange("p (j h) -> p j h", h=NHEAD)
                    .to_broadcast([128, nst, NHEAD, GCLS]),
                    op=OP.mult,
                )
                ow_tiles[s_] = ow

            def scores(s_):
                # PE: bias outer-product + 2 accumulating MMs per tile
                # (rhs = [V | 0.2V] interleaved (h,t)), all FWL weight loads.
                # ACT: exp of both columns; DVE: max over t. (exp is monotone
                # so exp(max(s, .2s)) == max(exp s, exp .2s) == leaky+exp.)
                ts0, nst = sup_range(s_)
                htr = htr_tiles.pop(s_)
                s_ps = ps.tile(
                    [128, s_tiles * 2 * NHEAD], f32, tag="sps", name="sps",
                    padded_shape=[128, 512],
                )
                nw = nst * 2 * NHEAD
                nc.tensor.matmul(
                    s_ps[:, :nw], lhsT=ones_row[:1], rhs=c_rep[:1, :nw],
                    start=True, stop=False, skip_group_check=True,
                )
                for j in range(nst):
                    sl = slice(j * 2 * NHEAD, (j + 1) * 2 * NHEAD)
                    nc.tensor.matmul(
                        s_ps[:, sl], lhsT=htr[:, j, 0], rhs=v_bf[0],
                        start=False, stop=False, skip_group_check=True,
                    )
                    nc.tensor.matmul(
                        s_ps[:, sl], lhsT=htr[:, j, 1], rhs=v_bf[1],
                        start=False, stop=True, skip_group_check=True,
                    )
                prep2 = sg.tile(
                    [128, s_tiles * 2 * NHEAD], bf16, tag="prep2", name="prep2"
                )
                prep = sg.tile([128, s_tiles * NHEAD], bf16, tag="prep", name="prep")
                nc.scalar.activation(prep2[:, :nw], s_ps[:, :nw], AF.Exp)
                nc.vector.tensor_reduce(
                    out=prep[:, : nst * NHEAD].rearrange(
                        "p (j h) -> p j h", h=NHEAD
                    ),
                    in_=prep2[:, :nw].rearrange(
                        "p (j h t) -> p j h t", h=NHEAD, t=2
                    ),
                    axis=mybir.AxisListType.X,
                    op=OP.max,
                )
                mask_ow(s_, prep)

            def transp(s_, pool_iter=None):
                # PE transpose-mode hn -> htr (bf16 PSUM), ACT/DVE batched
                # copies to SBUF. pool_iter: generator yielding pooling MMs of
                # the PREVIOUS supertile, interleaved per 4-tile batch so the
                # copy engines get slack between PE transpose bursts.
                ts0, nst = sup_range(s_)
                hns = hns_tiles[s_]
                htr = htp.tile(
                    [128, s_tiles, 2, 128], bf16, tag="htr", name="htr"
                )
                cdma = htr_c[s_]
                if cdma > 0:
                    off = htr_off[s_]
                    nc.scalar.dma_start(
                        out=htr[:, 0:cdma], in_=htrd[:, off : off + cdma]
                    )
                for b0 in range(cdma, nst, 4):
                    b1 = min(b0 + 4, nst)
                    nb = b1 - b0
                    tp = trp.tile([128, 8, 128], bf16, tag="tp", name="tp")
                    for j in range(b0, b1):
                        for c in (0, 1):
                            nc.tensor.transpose(
                                tp[:, (j - b0) * 2 + c],
                                hns[:, j, c * 128 : (c + 1) * 128],
                                ident[:],
                            )
                    src = tp[:, : nb * 2].rearrange("p (t c) f -> p t c f", c=2)
                    dst = htr[:, b0:b1]
                    if copy_rr["i"] % 2 == 1:
                        nc.scalar.activation(dst, src, AF.Copy)
                    else:
                        nc.vector.tensor_copy(out=dst, in_=src)
                    copy_rr["i"] += 1
                    if pool_iter is not None:
                        for _ in range(4):
                            if next(pool_iter, None) is None:
                                pool_iter = None
                                break
                htr_tiles[s_] = htr
                return pool_iter

            def pool_mms(s_):
                # generator: per tile 2 scatter-add MMs (hn chunks stationary,
                # ow moving); denom MMs hoisted (ones_col stays stationary);
                # group flushes + output DMA batched per OBATCH groups
                ts0, nst = sup_range(s_)
                hns = hns_tiles.pop(s_)
                ow = ow_tiles.pop(s_)
                accs = []
                ended = []
                for j in range(nst):
                    t_ = ts0 + j
                    g, first, last = grp_of[t_]
                    if first:
                        acc_state["t"] = accp.tile(
                            [128, OUTW], f32, tag="gacc", name="gacc",
                            padded_shape=[128, 512],
                        )
                    acc = acc_state["t"]
                    accs.append(acc)
                    rhs = ow[:, j].rearrange("p h w -> p (h w)")
                    nc.tensor.matmul(
                        acc[:, 0:HWW], lhsT=hns[:, j, 0:128], rhs=rhs,
                        start=first, stop=last, skip_group_check=True,
                    )
                    nc.tensor.matmul(
                        acc[:, HWW : 2 * HWW], lhsT=hns[:, j, 128:256],
                        rhs=rhs, start=False, stop=last,
                        skip_group_check=True,
                    )
                    if last:
                        ended.append((g, acc))
                    yield j
                for j in range(nst):
                    t_ = ts0 + j
                    g, first, last = grp_of[t_]
                    nc.tensor.matmul(
                        accs[j][0:1, 2 * HWW : OUTW], lhsT=ones_col[:],
                        rhs=ow[:, j].rearrange("p h w -> p (h w)"),
                        start=False, stop=last, skip_group_check=True,
                    )
                for g, acc in ended:
                    nc.scalar.activation(
                        out_sb[:, g * OUTW : (g + 1) * OUTW],
                        acc[:, 0:OUTW], AF.Copy,
                    )
                    if g % OBATCH == OBATCH - 1 or g == NGRP - 1:
                        g0 = (g // OBATCH) * OBATCH
                        nc.scalar.dma_start(
                            out=out[:, g0 * OUTW : (g + 1) * OUTW],
                            in_=out_sb[:, g0 * OUTW : (g + 1) * OUTW],
                        )

            # ---- software pipeline -----------------------------------------
            la = 5
            for s_ in range(min(la, nsup)):
                load(s_)
            transp(0)
            for s_ in range(nsup):
                scores(s_)
                if s_ + la < nsup:
                    load(s_ + la)
                pit = pool_mms(s_)
                if s_ + 1 < nsup:
                    pit = transp(s_ + 1, pit)
                if pit is not None:
                    for _ in pit:
                        pass

    nc.compile()
    return nc


def _place_classes(counts):
    """Bin-pack classes into (core, group, slot): LPT across cores, then
    snake-deal + swap-repair into NGRP bins of <=GCLS classes per core,
    pushing every bin under GCAP nodes (9 tiles) so the SPMD per-group
    tile counts (max across cores) carry minimal padding."""
    order = np.argsort(-counts, kind="stable")
    core_tot = np.zeros(NCORES, dtype=np.int64)
    core_cls = [[] for _ in range(NCORES)]
    for c in order:
        for k in sorted(range(NCORES), key=lambda i: int(core_tot[i])):
            if len(core_cls[k]) < NGRP * GCLS:
                break
        core_cls[k].append(int(c))
        core_tot[k] += counts[c]

    placement = []
    for k in range(NCORES):
        cls = sorted(core_cls[k], key=lambda c: -int(counts[c]))
        bins = [[] for _ in range(NGRP)]
        g, d = 0, 1
        for c in cls:
            bins[g].append(c)
            g += d
            if g == NGRP:
                g, d = NGRP - 1, -1
            elif g < 0:
                g, d = 0, 1
        bsum = np.array([sum(int(counts[c]) for c in b) for b in bins])
        for _ in range(500):
            hi = int(np.argmax(bsum))
            if bsum[hi] <= GCAP:
                break
            done = False
            for lo in np.argsort(bsum):
                if lo == hi:
                    continue
                for c1 in sorted(bins[hi], key=lambda c: -int(counts[c])):
                    for c2 in sorted(bins[lo], key=lambda c: int(counts[c])):
                        dd = int(counts[c1]) - int(counts[c2])
                        if dd > 0 and bsum[lo] + dd <= GCAP:
                            bins[hi].remove(c1)
                            bins[lo].remove(c2)
                            bins[hi].append(c2)
                            bins[lo].append(c1)
                            bsum[hi] -= dd
                            bsum[lo] += dd
                            done = True
                            break
                    if done:
                        break
                if done:
                    break
            if not done:
                break
        rank = np.argsort(-bsum, kind="stable")
        placement.append([bins[int(g)] for g in rank])
    return placement


def _prep_inputs(context_h, W_lin, b_lin, W_att, b_att, context_y):
    """Host-side shard: bin-pack classes across cores/groups, gather each
    bin's nodes, pad bins to whole 128-node tiles."""
    h = np.asarray(context_h, dtype=np.float32)
    hb = h.astype(BF)
    y = np.asarray(context_y).astype(np.int64)
    counts = np.bincount(y, minlength=NCLS)
    placement = _place_classes(counts)

    # node indices per class (sorted once)
    order = np.argsort(y, kind="stable")
    ys = y[order]
    cbounds = np.searchsorted(ys, np.arange(NCLS + 1))

    cnts = np.zeros((NCORES, NGRP), dtype=np.int64)
    for k in range(NCORES):
        for g in range(NGRP):
            cnts[k, g] = sum(int(counts[c]) for c in placement[k][g])
    tg = tuple(int(-(-int(cnts[:, g].max()) // 128)) for g in range(NGRP))
    t_tiles = int(sum(tg))

    # fold V[k,h], c[h] from W_lin/W_att/b_lin/b_att (weights-only preproc)
    W_lin = np.asarray(W_lin, dtype=np.float32)
    W_att = np.asarray(W_att, dtype=np.float32)
    V = np.einsum("hok,o->kh", W_lin.reshape(NHEAD, OUTC, INC), W_att)  # [256,4]
    c = (
        np.asarray(b_lin, dtype=np.float32).reshape(NHEAD, OUTC) @ W_att
        + np.asarray(b_att, dtype=np.float32)[0]
    )  # [4]
    cst = np.zeros((128, 192), dtype=BF)
    cst[:, 0:GCLS] = np.arange(GCLS, dtype=np.float32)[None, :].astype(BF)
    # [V | 0.2V] interleaved (h, t): col 2h = V[:,h], col 2h+1 = 0.2V[:,h]
    v8 = np.stack([V, 0.2 * V], axis=2).reshape(INC, 2 * NHEAD)
    cst[:, 32:40] = v8[0:128].astype(BF)
    cst[:, 40:48] = v8[128:256].astype(BF)
    c8 = np.stack([c, 0.2 * c], axis=1).reshape(2 * NHEAD)
    cst[0, 48:56] = c8.astype(BF)
    cst[:, 64:192] = np.eye(128, dtype=np.float32).astype(BF)

    in_maps = []
    for k in range(NCORES):
        hp = np.zeros((t_tiles * 128, NCOL), dtype=BF)
        hp[:, INC] = BF(100.0)  # pad marker -> mask==0
        row = 0
        for g in range(NGRP):
            for w, c in enumerate(placement[k][g]):
                lo, hi = cbounds[c], cbounds[c + 1]
                cnt = hi - lo
                hp[row : row + cnt, 0:INC] = hb[order[lo:hi]]
                hp[row : row + cnt, INC] = BF(float(w))
                row += cnt
            assert row <= sum(tg[: g + 1]) * 128
            row = sum(tg[: g + 1]) * 128
        hn = np.ascontiguousarray(
            hp.reshape(t_tiles, 128, NCOL).transpose(1, 0, 2)
        )
        # feature-major copies for the DMA-sourced htr tiles (if any)
        sizes, htr_c = _schedule(t_tiles)
        m = {"hn": hn, "cst": cst}
        if sum(htr_c) > 0:
            dt_list = []
            ts0 = 0
            for s, sz in enumerate(sizes):
                dt_list.extend(range(ts0, ts0 + htr_c[s]))
                ts0 += sz
            hpt = hp[:, 0:INC].reshape(t_tiles, 128, 2, 128)
            m["htrd"] = np.ascontiguousarray(
                hpt[dt_list].transpose(3, 0, 2, 1)
            )  # [128 feat, ct, chunk, node]
        in_maps.append(m)
    return in_maps, tg, placement


def kernel(context_h, W_lin, b_lin, W_att, b_att, context_y, num_classes):
    global LAST_RESULT, LAST_PROFILE
    import os

    assert int(num_classes) == NCLS

    from concourse.bass_utils import run_bass_kernel_spmd

    in_maps, tg, placement = _prep_inputs(
        context_h, W_lin, b_lin, W_att, b_att, context_y
    )
    if tg not in _PROG_CACHE:
        _PROG_CACHE[tg] = build_program(tg)
    nc = _PROG_CACHE[tg]
    core_ids = list(range(NCORES))
    res = run_bass_kernel_spmd(nc, in_maps, core_ids)
    LAST_RESULT = res

    if os.environ.get("KERNEL_PROFILE") == "1":
        LAST_PROFILE = run_bass_kernel_spmd(nc, in_maps, core_ids, trace=True)

    outp = np.empty((NCLS, NHEAD * INC), dtype=np.float32)
    for k in range(NCORES):
        o = np.asarray(res.results[k]["outp"])  # [128, NGRP*OUTW]
        for g in range(NGRP):
            blk = o[:, g * OUTW : (g + 1) * OUTW]
            c0 = blk[:, 0:HWW].reshape(128, NHEAD, GCLS)
            c1 = blk[:, HWW : 2 * HWW].reshape(128, NHEAD, GCLS)
            den = blk[0, 2 * HWW : OUTW].reshape(NHEAD, GCLS)
            den = np.where(den != 0.0, den, 1.0)
            pooled = np.concatenate([c0, c1], axis=0)  # [256, h, w]
            pooled = pooled / den[None, :, :]
            pw = pooled.transpose(2, 1, 0).reshape(GCLS, NHEAD * INC)
            for w, c in enumerate(placement[k][g]):
                outp[c] = pw[w]
    return outp
